# revision 1
# baseline (speedup 1.0000x reference)
"""GRU decoder kernel for Trainium2 (8 NeuronCores, SPMD data-parallel).

Problem: nn_Decoder — embedding lookup -> 256-step GRU -> vocab projection.
  B=16, T=256, H=1024, E=512, V=32000.

Sharding: data-parallel over batch (2 rows per core). All weights replicated.
No cross-core communication (collectives measured ~400us each here — any
per-step exchange is far slower than replicating the recurrence matmul).

Per-core layouts:
  tokens: tok = b*256 + t  (b in 0..2 local batch rows)
  h-state "column packed": [128 part, 16] with col = 2*k + b, value h[b, k*128+p]
  gate-major psum [128, 48]: col = m*2 + b where m = gate*8 + k  (gate r,z,n)
  xw SBUF [128, T*48]: col = t*48 + m*2 + b  (input-gate preactivations + bi)
  hsT SBUF [128, 8*512] bf16: col = k*512 + b*256 + t  (h AFTER step t)
"""

import os
import sys
import time
from contextlib import ExitStack

import numpy as np

sys.path.insert(0, "/opt/trn_rl_repo")

import concourse.bass as bass
import concourse.tile as tile
from concourse import bacc, mybir
from concourse.bass import IndirectOffsetOnAxis
from concourse.masks import make_identity

F32 = mybir.dt.float32
BF16 = mybir.dt.bfloat16
I32 = mybir.dt.int32
AF = mybir.ActivationFunctionType
OP = mybir.AluOpType

B, T, H, E, V = 16, 256, 1024, 512, 32000
NCORES = 8
BPC = B // NCORES          # batch rows per core = 2
TOK = BPC * T              # tokens per core = 512
KE = 5                     # E k-tiles incl. aug row block (640 = 5*128)
KH = 8                     # H k-tiles
M3 = 24                    # gate-col blocks (3H/128)
EA = KE * 128              # 640
HA = 1152                  # 9*128 (H + aug ones row + pad)


def build(T_steps=T, reps=1):
    nc = bacc.Bacc("TRN2", target_bir_lowering=False, debug=False,
                   num_devices=NCORES)

    tgt = nc.dram_tensor("tgt", [TOK, 1], I32, kind="ExternalInput")
    h0T = nc.dram_tensor("h0T", [128, 2 * KH], F32, kind="ExternalInput")
    emb = nc.dram_tensor("emb", [V, E], BF16, kind="ExternalInput")
    wh = nc.dram_tensor("wh", [H, 3 * H], BF16, kind="ExternalInput")
    wiA = nc.dram_tensor("wiA", [EA, 3 * H], BF16, kind="ExternalInput")
    wo = nc.dram_tensor("wo", [H, V], BF16, kind="ExternalInput")
    bo_in = nc.dram_tensor("bo_in", [1, V], F32, kind="ExternalInput")
    bhnT = nc.dram_tensor("bhnT", [128, 2 * KH], F32, kind="ExternalInput")
    out = nc.dram_tensor("out", [TOK, V], F32, kind="ExternalOutput")

    NV = 64                    # vocab n-chunks
    VC = V // NV               # 500 cols per chunk

    with tile.TileContext(nc) as tc:
        with ExitStack() as ctx:
            const = ctx.enter_context(tc.tile_pool(name="const", bufs=1))
            big = ctx.enter_context(tc.tile_pool(name="big", bufs=1))

            ident = const.tile([128, 128], BF16)
            make_identity(nc, ident[:])
            bhn_sb = const.tile([128, 2 * KH], F32)
            nc.sync.dma_start(bhn_sb[:], bhnT[:])
            h0bf = const.tile([128, 2 * KH], BF16)
            h0f = const.tile([128, 2 * KH], F32)
            nc.sync.dma_start(h0f[:], h0T[:])
            nc.vector.tensor_copy(h0bf[:], h0f[:])
            # resident weights
            wh_sb = big.tile([128, KH * 3 * H], BF16)
            for k in range(KH):
                nc.sync.dma_start(wh_sb[:, k * 3 * H:(k + 1) * 3 * H],
                                  wh[k * 128:(k + 1) * 128, :])

            xw_sb = big.tile([128, T_steps * 48], F32)
            hsT_sb = big.tile([128, KH * TOK], BF16)
            h_state = const.tile([128, 2 * KH], F32)

            for rep in range(reps):
                # ---------------- Phase A: embed gather + x^T + xW ----------
                with tc.tile_pool(name="phA", bufs=1) as phA, \
                     tc.tile_pool(name="xgp", bufs=2) as xgp, \
                     tc.tile_pool(name="idxp", bufs=2) as idxp, \
                     tc.tile_pool(name="psA", bufs=2, space="PSUM") as psA, \
                     tc.tile_pool(name="psT", bufs=2, space="PSUM") as psT:
                    wi_sb = phA.tile([128, KE * 3 * H], BF16)
                    for k in range(KE):
                        nc.sync.dma_start(wi_sb[:, k * 3 * H:(k + 1) * 3 * H],
                                          wiA[k * 128:(k + 1) * 128, :])
                    xT_sb = phA.tile([128, KE * TOK], BF16)
                    # aug k-block: ones row (partition 0), zeros elsewhere
                    nc.vector.memset(xT_sb[:, 4 * TOK:5 * TOK], 0.0)
                    nc.vector.memset(xT_sb[0:1, 4 * TOK:5 * TOK], 1.0)
                    for c in range(TOK // 128):
                        idx = idxp.tile([128, 1], I32)
                        nc.sync.dma_start(idx[:], tgt[c * 128:(c + 1) * 128, :])
                        xg = xgp.tile([128, E], BF16)
                        nc.gpsimd.indirect_dma_start(
                            out=xg[:], out_offset=None, in_=emb[:],
                            in_offset=IndirectOffsetOnAxis(ap=idx[:, :1], axis=0),
                        )
                        for eb in range(4):
                            pst = psT.tile([128, 128], BF16)
                            nc.tensor.transpose(pst[:],
                                                xg[:, eb * 128:(eb + 1) * 128],
                                                ident[:])
                            nc.vector.tensor_copy(
                                xT_sb[:, eb * TOK + c * 128: eb * TOK + (c + 1) * 128],
                                pst[:])

                    # xW[tok, 3H] in gate-major column-packed layout
                    xw_view = xw_sb[:].rearrange("p (t g) -> p g t", g=48)
                    for m in range(M3):
                        ps = psA.tile([128, TOK], F32)
                        for kb in range(KE):
                            nc.tensor.matmul(
                                ps[:],
                                lhsT=wi_sb[:, kb * 3 * H + m * 128: kb * 3 * H + (m + 1) * 128],
                                rhs=xT_sb[:, kb * TOK:(kb + 1) * TOK],
                                start=(kb == 0), stop=(kb == KE - 1))
                        dst = xw_view[:, m * 2:m * 2 + 2, :]
                        src = ps[:].rearrange("p (b t) -> p b t", b=2)[:, :, :T_steps]
                        nc.vector.tensor_copy(dst, src)

                # ---------------- Phase B: GRU recurrence -------------------
                hsT_view = hsT_sb[:].rearrange("p (k b t) -> p k b t", k=KH, b=2)
                if T_steps != T:
                    nc.vector.memset(hsT_sb[:], 0.0)

                def rhs_k(t, k):
                    if t == 0:
                        return h0bf[:, 2 * k:2 * k + 2]
                    return hsT_view[:, k, :, t - 1]

                with tc.tile_pool(name="gp", bufs=3) as gp, \
                     tc.tile_pool(name="wop", bufs=3) as wop, \
                     tc.tile_pool(name="bop", bufs=2) as bop, \
                     tc.tile_pool(name="otp", bufs=3) as otp, \
                     tc.tile_pool(name="psP", bufs=2, space="PSUM") as psPp, \
                     tc.tile_pool(name="psH", bufs=2, space="PSUM") as psH:

                    def load_wo_chunk(n):
                        wt = wop.tile([128, KH * VC], BF16, tag="wo")
                        for k in range(KH):
                            nc.sync.dma_start(
                                wt[:, k * VC:(k + 1) * VC],
                                wo[k * 128:(k + 1) * 128, n * VC:(n + 1) * VC])
                        b1 = bop.tile([1, VC], F32, tag="b1")
                        nc.sync.dma_start(b1[:], bo_in[0:1, n * VC:(n + 1) * VC])
                        bbc = bop.tile([128, VC], F32, tag="bbc")
                        nc.gpsimd.partition_broadcast(bbc[:], b1[:])
                        return wt, bbc

                    def emit_proj_tile(tb, n, wt, bbc):
                        # out rows [tb*128:(tb+1)*128] = hsT token block tb
                        psP = psPp.tile([128, VC], F32)
                        for k in range(KH):
                            nc.tensor.matmul(
                                psP[:],
                                lhsT=hsT_sb[:, k * TOK + tb * 128: k * TOK + (tb + 1) * 128],
                                rhs=wt[:, k * VC:(k + 1) * VC],
                                start=(k == 0), stop=(k == KH - 1))
                        ot = otp.tile([128, VC], F32)
                        nc.vector.tensor_tensor(ot[:], psP[:], bbc[:], OP.add)
                        nc.sync.dma_start(
                            out[tb * 128:(tb + 1) * 128, n * VC:(n + 1) * VC], ot[:])

                    interleave = (T_steps == T)
                    wt_cur = None
                    for t in range(T_steps):
                        ps = psH.tile([128, 48], F32)
                        for m in range(M3):
                            for k in range(KH):
                                nc.tensor.matmul(
                                    ps[:, m * 2:(m + 1) * 2],
                                    lhsT=wh_sb[:, k * 3 * H + m * 128: k * 3 * H + (m + 1) * 128],
                                    rhs=rhs_k(t, k),
                                    start=(k == 0), stop=(k == KH - 1))
                        xwt = xw_sb[:, t * 48:(t + 1) * 48]
                        arz = gp.tile([128, 32], F32)
                        nc.vector.tensor_tensor(arz[:], ps[:, 0:32], xwt[:, 0:32], OP.add)
                        rz = gp.tile([128, 32], F32)
                        nc.scalar.activation(rz[:], arz[:], AF.Sigmoid)
                        hnb = gp.tile([128, 16], F32)
                        nc.vector.tensor_tensor(hnb[:], ps[:, 32:48], bhn_sb[:], OP.add)
                        rn = gp.tile([128, 16], F32)
                        nc.vector.tensor_tensor(rn[:], rz[:, 0:16], hnb[:], OP.mult)
                        an = gp.tile([128, 16], F32)
                        nc.vector.tensor_tensor(an[:], rn[:], xwt[:, 32:48], OP.add)
                        n_ = gp.tile([128, 16], F32)
                        nc.scalar.activation(n_[:], an[:], AF.Tanh)
                        hprev = h0f if t == 0 else h_state
                        d = gp.tile([128, 16], F32)
                        nc.vector.tensor_tensor(d[:], hprev[:], n_[:], OP.subtract)
                        zd = gp.tile([128, 16], F32)
                        nc.vector.tensor_tensor(zd[:], rz[:, 16:32], d[:], OP.mult)
                        nc.vector.tensor_tensor(h_state[:], n_[:], zd[:], OP.add)
                        nc.vector.tensor_copy(
                            hsT_view[:, :, :, t],
                            h_state[:].rearrange("p (k b) -> p k b", k=KH))

                        # interleave first-half projection (token blocks 0, 2:
                        # t<128 for both batch rows) into steps 128..255
                        if interleave and t >= T // 2:
                            i = t - T // 2
                            if i % 2 == 0:
                                wt_cur = load_wo_chunk(i // 2)
                            emit_proj_tile((i % 2) * 2, i // 2, *wt_cur)

                    # ---------- projection tail (token blocks 1, 3) --------
                    tail = ([1, 3] if interleave else [0, 1, 2, 3])
                    for n in range(NV):
                        wt, bbc = load_wo_chunk(n)
                        for tb in tail:
                            emit_proj_tile(tb, n, wt, bbc)
                if rep != reps - 1:
                    tc.strict_bb_all_engine_barrier()

    nc.compile()
    return nc


# ---------------------------------------------------------------------------
# host side
# ---------------------------------------------------------------------------

def _pack_colmajor(vec_2d, bpc_rows):
    """[bpc, H] f32 -> [128, 2*KH] with col = 2*k + b."""
    o = np.zeros((128, 2 * KH), np.float32)
    for k in range(KH):
        for b in range(bpc_rows):
            o[:, 2 * k + b] = vec_2d[b, k * 128:(k + 1) * 128]
    return o


def make_in_maps(encoder_state, targets, embed_table, Wi, Wh, bi, bhn, Wo, bo):
    encoder_state = np.asarray(encoder_state, np.float32)
    targets = np.asarray(targets)
    embed_table = np.asarray(embed_table, np.float32)
    Wi = np.asarray(Wi, np.float32)
    Wh = np.asarray(Wh, np.float32)
    bi = np.asarray(bi, np.float32)
    bhn = np.asarray(bhn, np.float32)
    Wo = np.asarray(Wo, np.float32)
    bo = np.asarray(bo, np.float32)

    import ml_dtypes
    emb_bf = embed_table.astype(ml_dtypes.bfloat16)
    wh_bf = Wh.astype(ml_dtypes.bfloat16)
    wiA = np.zeros((EA, 3 * H), np.float32)
    wiA[:E] = Wi
    wiA[E] = bi
    wiA_bf = wiA.astype(ml_dtypes.bfloat16)
    wo_bf = Wo.astype(ml_dtypes.bfloat16)
    bo_row = bo.reshape(1, V).astype(np.float32)

    bhn_pack = _pack_colmajor(np.broadcast_to(bhn, (BPC, H)), BPC)

    in_maps = []
    for c in range(NCORES):
        rows = slice(c * BPC, (c + 1) * BPC)
        tgt = targets[rows].reshape(TOK, 1).astype(np.int32)
        h0 = _pack_colmajor(encoder_state[rows], BPC)
        in_maps.append({
            "tgt": tgt,
            "h0T": h0,
            "emb": emb_bf,
            "wh": wh_bf,
            "wiA": wiA_bf,
            "wo": wo_bf,
            "bo_in": bo_row,
            "bhnT": bhn_pack,
        })
    return in_maps


_NC_CACHE = {}


def get_nc(T_steps=T, reps=1):
    key = (T_steps, reps)
    if key not in _NC_CACHE:
        _NC_CACHE[key] = build(T_steps, reps)
    return _NC_CACHE[key]


def kernel(encoder_state, targets, embed_table, Wi, Wh, bi, bhn, Wo, bo):
    from concourse.bass_utils import run_bass_kernel_spmd
    nc = get_nc()
    in_maps = make_in_maps(encoder_state, targets, embed_table, Wi, Wh, bi,
                           bhn, Wo, bo)
    res = run_bass_kernel_spmd(nc, in_maps, list(range(NCORES)))
    outs = [res.results[c]["out"].reshape(BPC, T, V) for c in range(NCORES)]
    return np.concatenate(outs, axis=0)

